# revision 20
# baseline (speedup 1.0000x reference)
"""EMA head kernel for Trainium2 (Bass/Tile), 8 NeuronCores.

Problem: alpha = clip(sigmoid(MLP(feat)), 0.01, 0.99) per (t, b);
         y[0] = r[0]; y[t] = (1-alpha[t])*y[t-1] + alpha[t]*r[t].

Sharding: time dim T=4096 split into 8 slabs of 512 (all B=256 per core).
Each core computes, for its slab, the local affine-scan pieces
    z[t] = A[t]*z[t-1] + Bv[t]   (z[-1] = 0),   A = 1-alpha, Bv = alpha*r
    P[t] = A[t]*P[t-1]           (P[-1] = 1)
and the host stitches slabs with   y = z + P * carry,  carry' = y[-1].

v7 design notes (measured on this platform):
- HBM aggregate across the 8 concurrent cores saturates at ~1.5 TB/s
  (~190 GB/s/core), independent of descriptor size / queue count, so the
  16 MB/core fp8 feat stream is a hard ~85 us floor; everything else
  must hide under it.
- feat is pre-transposed + quantized to fp8 e4m3 on the host:
  featT [2 (b-half j), 128 (f), TLOC (t), 128 (b)].  Uniform 1 MB chunks
  (64 t) with a DEDICATED SBUF tile per chunk: all DMAs issue up front,
  no buffer-reuse stalls, PE starts after ~5 us and never lags.
- W2 is folded into W1/b1 on the host (relu(x)*|w2| = relu(x*|w2|),
  columns permuted positives-first) so layer 2 is just two sub-range
  reduces and a subtract -- no elementwise multiply pass.
- b1 is pre-added into PSUM with a K=1 ones-matmul per 2-bank group
  (one per chunk), so ACT's relu reads matmul output directly.
- alpha/scan tail is processed in 128-t blocks with chained
  tensor_tensor_scan; z/P written out as fp16.
"""

import numpy as np

T, B, FEAT, HID = 4096, 256, 128, 16
NCORES = 8
TLOC = T // NCORES  # 512
NH = 2              # batch halves of 128
CH = 128            # t-steps per feat DMA chunk (2 MB fp8, 4 PSUM banks)
BANK = 32           # t-slots per PSUM bank
TB = 128            # t-steps per alpha/scan block

FEAT_FP8 = True
# W1 columns scaled by |W2| land in fp8-e4m3's subnormal range (<2^-6);
# pre-scale by 64 on the host and undo via the sigmoid's scale param.
WSCALE = 64.0

_CACHE = {}


def _np_feat_dtype():
    if FEAT_FP8:
        import ml_dtypes
        return ml_dtypes.float8_e4m3
    return np.float16


def _build_program(npos):
    import concourse.bacc as bacc
    import concourse.bass as bass
    import concourse.tile as tile
    from concourse import mybir

    fp32 = mybir.dt.float32
    fp16 = mybir.dt.float16
    fdt = mybir.dt.float8e4 if FEAT_FP8 else mybir.dt.float16
    AF = mybir.ActivationFunctionType
    OP = mybir.AluOpType

    nc = bacc.Bacc("TRN2", target_bir_lowering=False, debug=False,
                   num_devices=NCORES)

    featT_d = nc.dram_tensor("featT", [NH, FEAT, TLOC, 128], fdt,
                             kind="ExternalInput")
    rT_d = nc.dram_tensor("rT", [NH, 128, TLOC], fp32, kind="ExternalInput")
    w1_d = nc.dram_tensor("w1", [FEAT, HID], fdt, kind="ExternalInput")
    # b1 arrives pre-folded, pre-replicated CH times, already fp16 (plain
    # DMA -- HWDGE cannot cast); b2 pre-replicated to 128 partitions.
    b1_d = nc.dram_tensor("b1rep", [BANK * HID], fp16, kind="ExternalInput")
    b2_d = nc.dram_tensor("b2rep", [128], fp32, kind="ExternalInput")
    z_d = nc.dram_tensor("z", [NH, 128, TLOC], fp16, kind="ExternalOutput")
    p_d = nc.dram_tensor("p", [NH, 128, TLOC], fp16, kind="ExternalOutput")

    with tile.TileContext(nc) as tc:
        with (
            tc.tile_pool(name="singles", bufs=1) as singles,
            tc.tile_pool(name="hps", bufs=2, space="PSUM") as hps,
            tc.tile_pool(name="hwork", bufs=3) as hwork,
            tc.tile_pool(name="apool", bufs=2) as apool,
        ):
            # feat chunk DMAs first: dedicated tiles, all issued up front
            # on the two big queues (sync HWDGE / gpsimd SWDGE).
            ft_tiles = {}
            parity = 0
            for j in range(NH):
                for ci in range(TLOC // CH):
                    t0 = ci * CH
                    ft = singles.tile([128, CH, 128], fdt, tag=f"ft{j}_{ci}",
                                      name=f"ft{j}_{ci}")
                    eng = nc.sync if parity == 0 else nc.gpsimd
                    parity ^= 1
                    eng.dma_start(ft, featT_d[j, :, t0:t0 + CH, :])
                    ft_tiles[(j, ci)] = ft

            # constants on the scalar (ACT) HWDGE queue -- nothing big
            # ever queues there, so they land in the first microseconds.
            w1_sb = singles.tile([128, HID], fdt)
            nc.scalar.dma_start(w1_sb, w1_d[:, :])
            b1row = singles.tile([1, BANK, HID], fp16)
            nc.scalar.dma_start(
                b1row, bass.AP(b1_d, 0, [[0, 1], [HID, BANK], [1, HID]]))
            b2col = singles.tile([128, 1], fp32)
            nc.scalar.dma_start(b2col, bass.AP(b2_d, 0, [[1, 128], [1, 1]]))
            rT = [singles.tile([128, TLOC], fp32, tag=f"rT{h}", name=f"rT{h}")
                  for h in range(NH)]
            for h in range(NH):
                nc.scalar.dma_start(rT[h], rT_d[h])
            ones1 = singles.tile([1, 128], fp16)
            nc.vector.memset(ones1, 1.0)
            ones_tb = singles.tile([128, TB], fp32)
            nc.vector.memset(ones_tb, 1.0)

            apre = [singles.tile([128, TLOC], fp32, tag=f"apre{h}",
                                 name=f"apre{h}")
                    for h in range(NH)]
            z_sb = [singles.tile([128, TLOC], fp16, tag=f"z{h}", name=f"z{h}")
                    for h in range(NH)]
            p_sb = [singles.tile([128, TLOC], fp16, tag=f"p{h}", name=f"p{h}")
                    for h in range(NH)]

            def do_block(j, blk):
                """alpha -> A,Bv -> chained scans for t in [blk, blk+TB)."""
                al = apool.tile([128, TB], fp32, tag="al")
                nc.scalar.activation(al, apre[j][:, blk:blk + TB],
                                     AF.Sigmoid, bias=b2col,
                                     scale=1.0 / WSCALE)
                nc.vector.tensor_scalar(al, al, 0.01, 0.99,
                                        op0=OP.max, op1=OP.min)
                A_b = apool.tile([128, TB], fp32, tag="A")
                nc.vector.tensor_scalar(A_b, al, -1.0, 1.0,
                                        op0=OP.mult, op1=OP.add)
                Bv = apool.tile([128, TB], fp32, tag="Bv")
                nc.vector.tensor_mul(Bv, al, rT[j][:, blk:blk + TB])
                z0 = 0.0 if blk == 0 else z_sb[j][:, blk - 1:blk]
                nc.vector.tensor_tensor_scan(
                    z_sb[j][:, blk:blk + TB], A_b, Bv, z0,
                    op0=OP.mult, op1=OP.add)
                p0 = 1.0 if blk == 0 else p_sb[j][:, blk - 1:blk]
                nc.vector.tensor_tensor_scan(
                    p_sb[j][:, blk:blk + TB], A_b, ones_tb, p0,
                    op0=OP.mult, op1=OP.mult)
                if blk + TB == TLOC:
                    nc.gpsimd.dma_start(z_d[j], z_sb[j])
                    nc.sync.dma_start(p_d[j], p_sb[j])

            for j in range(NH):
                for ci in range(TLOC // CH):
                    t0 = ci * CH
                    ft = ft_tiles[(j, ci)]
                    # one 2-bank PSUM group per chunk: bias preload via
                    # K=1 ones-matmul, then 64 accumulating per-t matmuls
                    hbank = hps.tile([128, CH, HID], fp32, tag="hb")
                    # one bias preload per PSUM bank (matmul output may
                    # not cross the 512-fp32 bank boundary)
                    for hb in range(CH // BANK):
                        nc.tensor.matmul(hbank[:, hb * BANK:(hb + 1) * BANK, :],
                                         ones1, b1row,
                                         start=True, stop=False)
                        for si in range(BANK):
                            s = hb * BANK + si
                            nc.tensor.matmul(hbank[:, s, :], ft[:, s, :],
                                             w1_sb, start=False,
                                             stop=(si == BANK - 1))
                    hrelu = hwork.tile([128, CH, HID], fp16, tag="hrelu")
                    nc.scalar.activation(hrelu, hbank, AF.Relu)
                    # layer 2 with W2 pre-folded into W1/b1: sum over the
                    # positive-sign columns minus sum over the negatives.
                    dst = apre[j][:, t0:t0 + CH]
                    if npos == HID:
                        nc.vector.tensor_reduce(
                            dst, hrelu, axis=mybir.AxisListType.X, op=OP.add)
                    elif npos == 0:
                        nc.vector.tensor_reduce(
                            dst, hrelu, axis=mybir.AxisListType.X, op=OP.add)
                        nc.vector.tensor_scalar_mul(dst, dst, -1.0)
                    else:
                        nc.vector.tensor_reduce(
                            dst, hrelu[:, :, 0:npos],
                            axis=mybir.AxisListType.X, op=OP.add)
                        neg = hwork.tile([128, CH], fp32, tag="neg")
                        nc.vector.tensor_reduce(
                            neg, hrelu[:, :, npos:HID],
                            axis=mybir.AxisListType.X, op=OP.add)
                        nc.vector.tensor_tensor(dst, dst, neg,
                                                op=OP.subtract)
                    if (t0 + CH) % TB == 0:
                        do_block(j, t0 + CH - TB)

    nc.finalize()
    return nc


def _get_program(npos):
    if npos not in _CACHE:
        _CACHE[npos] = _build_program(npos)
    return _CACHE[npos]


def kernel(r, feat, W1, b1, W2, b2, _run_kwargs=None, _return_results=False):
    from concourse.bass_utils import run_bass_kernel_spmd

    fdt = _np_feat_dtype()
    r = np.asarray(r, dtype=np.float32)
    feat = np.asarray(feat, dtype=np.float32)
    W1 = np.asarray(W1, dtype=np.float32)
    b1 = np.asarray(b1, dtype=np.float32).reshape(HID)
    W2 = np.asarray(W2, dtype=np.float32).reshape(HID)
    b2 = np.asarray(b2, dtype=np.float32).reshape(1)

    # fold W2 into W1/b1: relu(x)*w2 = sign(w2)*relu(x*|w2|); reorder
    # hidden columns so positive-sign ones come first.
    s = np.abs(W2)
    neg_mask = W2 < 0
    order = np.argsort(neg_mask, kind="stable")
    npos = int((~neg_mask).sum())
    W1f = (W1 * s[None, :])[:, order] * WSCALE
    b1f = (b1 * s)[order] * WSCALE

    # host-side downcast + transpose: [T,B,F] -> [core, j, f, t_loc, b]
    featT = np.ascontiguousarray(
        feat.astype(fdt).reshape(NCORES, TLOC, NH, 128, FEAT)
            .transpose(0, 2, 4, 1, 3))
    # r: [T,B,1] -> [core, j, b, t_loc]
    rT = np.ascontiguousarray(
        r[:, :, 0].reshape(NCORES, TLOC, NH, 128).transpose(0, 2, 3, 1))
    w1c = np.ascontiguousarray(W1f.astype(fdt))

    b1rep = np.ascontiguousarray(np.tile(b1f, BANK).astype(np.float16))
    b2rep = np.ascontiguousarray(np.full(128, b2[0], dtype=np.float32))

    nc = _get_program(npos)
    in_maps = []
    for c in range(NCORES):
        in_maps.append({
            "featT": featT[c], "rT": rT[c],
            "w1": w1c, "b1rep": b1rep, "b2rep": b2rep,
        })

    kw = _run_kwargs or {}
    res = run_bass_kernel_spmd(nc, in_maps, core_ids=list(range(NCORES)), **kw)

    # host stitch: y = z + P*carry per slab, carry chain across slabs
    y = np.empty((T, B), dtype=np.float32)
    carry = r[0, :, 0].astype(np.float32)
    for c in range(NCORES):
        zc = res.results[c]["z"].astype(np.float32).transpose(2, 0, 1)
        pc = res.results[c]["p"].astype(np.float32).transpose(2, 0, 1)
        zc = zc.reshape(TLOC, B)
        pc = pc.reshape(TLOC, B)
        y_slab = zc + pc * carry[None, :]
        carry = y_slab[-1]
        y[c * TLOC:(c + 1) * TLOC] = y_slab
    out = y[:, :, None]
    if _return_results:
        return out, res
    return out


# revision 21
# speedup vs baseline: 1.0903x; 1.0903x over previous
"""EMA head kernel for Trainium2 (Bass/Tile), 8 NeuronCores.

Problem: alpha = clip(sigmoid(MLP(feat)), 0.01, 0.99) per (t, b);
         y[0] = r[0]; y[t] = (1-alpha[t])*y[t-1] + alpha[t]*r[t].

Sharding: time dim T=4096 split into 8 slabs of 512 (all B=256 per core).
Each core computes, for its slab, the local affine-scan pieces
    z[t] = A[t]*z[t-1] + Bv[t]   (z[-1] = 0),   A = 1-alpha, Bv = alpha*r
    P[t] = A[t]*P[t-1]           (P[-1] = 1)
and the host stitches slabs with   y = z + P * carry,  carry' = y[-1].

v7 design notes (measured on this platform):
- HBM aggregate across the 8 concurrent cores saturates at ~1.5 TB/s
  (~190 GB/s/core), independent of descriptor size / queue count, so the
  16 MB/core fp8 feat stream is a hard ~85 us floor; everything else
  must hide under it.
- feat is pre-transposed + quantized to fp8 e4m3 on the host:
  featT [2 (b-half j), 128 (f), TLOC (t), 128 (b)].  Uniform 1 MB chunks
  (64 t) with a DEDICATED SBUF tile per chunk: all DMAs issue up front,
  no buffer-reuse stalls, PE starts after ~5 us and never lags.
- W2 is folded into W1/b1 on the host (relu(x)*|w2| = relu(x*|w2|),
  columns permuted positives-first) so layer 2 is just two sub-range
  reduces and a subtract -- no elementwise multiply pass.
- b1 is pre-added into PSUM with a K=1 ones-matmul per 2-bank group
  (one per chunk), so ACT's relu reads matmul output directly.
- alpha/scan tail is processed in 128-t blocks with chained
  tensor_tensor_scan; z/P written out as fp16.
"""

import numpy as np

T, B, FEAT, HID = 4096, 256, 128, 16
NCORES = 8
TLOC = T // NCORES  # 512
NH = 2              # batch halves of 128
# per-half feat chunk schedule: small first chunk so PE starts early,
# big 2MB middle chunks for descriptor efficiency (16KB/partition),
# small final chunks to shrink the serial pipeline tail.
CHUNKS = [64, 128, 128, 128, 32, 32]
BANK = 32           # t-slots per PSUM bank
TB = 128            # t-steps per alpha/scan block

FEAT_FP8 = True
# W1 columns scaled by |W2| land in fp8-e4m3's subnormal range (<2^-6);
# pre-scale by 64 on the host and undo via the sigmoid's scale param.
WSCALE = 64.0

_CACHE = {}


def _np_feat_dtype():
    if FEAT_FP8:
        import ml_dtypes
        return ml_dtypes.float8_e4m3
    return np.float16


def _build_program(npos):
    import concourse.bacc as bacc
    import concourse.bass as bass
    import concourse.tile as tile
    from concourse import mybir

    fp32 = mybir.dt.float32
    fp16 = mybir.dt.float16
    fdt = mybir.dt.float8e4 if FEAT_FP8 else mybir.dt.float16
    AF = mybir.ActivationFunctionType
    OP = mybir.AluOpType

    nc = bacc.Bacc("TRN2", target_bir_lowering=False, debug=False,
                   num_devices=NCORES)

    featT_d = nc.dram_tensor("featT", [NH, FEAT, TLOC, 128], fdt,
                             kind="ExternalInput")
    rT_d = nc.dram_tensor("rT", [NH, 128, TLOC], fp32, kind="ExternalInput")
    w1_d = nc.dram_tensor("w1", [FEAT, HID], fdt, kind="ExternalInput")
    # b1 arrives pre-folded, pre-replicated CH times, already fp16 (plain
    # DMA -- HWDGE cannot cast); b2 pre-replicated to 128 partitions.
    b1_d = nc.dram_tensor("b1rep", [BANK * HID], fp16, kind="ExternalInput")
    b2_d = nc.dram_tensor("b2rep", [128], fp32, kind="ExternalInput")
    z_d = nc.dram_tensor("z", [NH, 128, TLOC], fp16, kind="ExternalOutput")
    p_d = nc.dram_tensor("p", [NH, 128, TLOC], fp16, kind="ExternalOutput")

    with tile.TileContext(nc) as tc:
        with (
            tc.tile_pool(name="singles", bufs=1) as singles,
            tc.tile_pool(name="hps", bufs=2, space="PSUM") as hps,
            tc.tile_pool(name="hwork", bufs=3) as hwork,
            tc.tile_pool(name="apool", bufs=2) as apool,
        ):
            # feat chunk DMAs first: dedicated tiles, all issued up front
            # on the two big queues (sync HWDGE / gpsimd SWDGE).
            ft_tiles = {}
            parity = 0
            for j in range(NH):
                t0 = 0
                for ci, ch in enumerate(CHUNKS):
                    ft = singles.tile([128, ch, 128], fdt, tag=f"ft{j}_{ci}",
                                      name=f"ft{j}_{ci}")
                    eng = nc.sync if parity == 0 else nc.gpsimd
                    parity ^= 1
                    eng.dma_start(ft, featT_d[j, :, t0:t0 + ch, :])
                    ft_tiles[(j, ci)] = ft
                    t0 += ch

            # constants on the scalar (ACT) HWDGE queue -- nothing big
            # ever queues there, so they land in the first microseconds.
            w1_sb = singles.tile([128, HID], fdt)
            nc.scalar.dma_start(w1_sb, w1_d[:, :])
            b1row = singles.tile([1, BANK, HID], fp16)
            nc.scalar.dma_start(
                b1row, bass.AP(b1_d, 0, [[0, 1], [HID, BANK], [1, HID]]))
            b2col = singles.tile([128, 1], fp32)
            nc.scalar.dma_start(b2col, bass.AP(b2_d, 0, [[1, 128], [1, 1]]))
            rT = [singles.tile([128, TLOC], fp32, tag=f"rT{h}", name=f"rT{h}")
                  for h in range(NH)]
            for h in range(NH):
                nc.scalar.dma_start(rT[h], rT_d[h])
            ones1 = singles.tile([1, 128], fp16)
            nc.vector.memset(ones1, 1.0)
            ones_tb = singles.tile([128, TB], fp32)
            nc.vector.memset(ones_tb, 1.0)

            apre = [singles.tile([128, TLOC], fp32, tag=f"apre{h}",
                                 name=f"apre{h}")
                    for h in range(NH)]
            z_sb = [singles.tile([128, TLOC], fp16, tag=f"z{h}", name=f"z{h}")
                    for h in range(NH)]
            p_sb = [singles.tile([128, TLOC], fp16, tag=f"p{h}", name=f"p{h}")
                    for h in range(NH)]

            def do_block(j, blk):
                """alpha -> A,Bv -> chained scans for t in [blk, blk+TB)."""
                al = apool.tile([128, TB], fp32, tag="al")
                nc.scalar.activation(al, apre[j][:, blk:blk + TB],
                                     AF.Sigmoid, bias=b2col,
                                     scale=1.0 / WSCALE)
                nc.vector.tensor_scalar(al, al, 0.01, 0.99,
                                        op0=OP.max, op1=OP.min)
                A_b = apool.tile([128, TB], fp32, tag="A")
                nc.vector.tensor_scalar(A_b, al, -1.0, 1.0,
                                        op0=OP.mult, op1=OP.add)
                Bv = apool.tile([128, TB], fp32, tag="Bv")
                nc.vector.tensor_mul(Bv, al, rT[j][:, blk:blk + TB])
                z0 = 0.0 if blk == 0 else z_sb[j][:, blk - 1:blk]
                nc.vector.tensor_tensor_scan(
                    z_sb[j][:, blk:blk + TB], A_b, Bv, z0,
                    op0=OP.mult, op1=OP.add)
                p0 = 1.0 if blk == 0 else p_sb[j][:, blk - 1:blk]
                nc.vector.tensor_tensor_scan(
                    p_sb[j][:, blk:blk + TB], A_b, ones_tb, p0,
                    op0=OP.mult, op1=OP.mult)
                done = blk + TB
                if done == TLOC // 2 or done == TLOC:
                    lo = 0 if done == TLOC // 2 else TLOC // 2
                    nc.gpsimd.dma_start(z_d[j][:, lo:done],
                                        z_sb[j][:, lo:done])
                    nc.sync.dma_start(p_d[j][:, lo:done],
                                      p_sb[j][:, lo:done])

            for j in range(NH):
                t0 = 0
                next_blk = 0
                for ci, ch in enumerate(CHUNKS):
                    ft = ft_tiles[(j, ci)]
                    # one 2-bank PSUM group per chunk: bias preload via
                    # K=1 ones-matmul, then 64 accumulating per-t matmuls
                    hbank = hps.tile([128, ch, HID], fp32, tag="hb")
                    # one bias preload per PSUM bank (matmul output may
                    # not cross the 512-fp32 bank boundary)
                    for hb in range(ch // BANK):
                        nc.tensor.matmul(hbank[:, hb * BANK:(hb + 1) * BANK, :],
                                         ones1, b1row,
                                         start=True, stop=False)
                        for si in range(BANK):
                            s = hb * BANK + si
                            nc.tensor.matmul(hbank[:, s, :], ft[:, s, :],
                                             w1_sb, start=False,
                                             stop=(si == BANK - 1))
                    hrelu = hwork.tile([128, ch, HID], fp16, tag="hrelu")
                    nc.scalar.activation(hrelu, hbank, AF.Relu)
                    # layer 2 with W2 pre-folded into W1/b1: sum over the
                    # positive-sign columns minus sum over the negatives.
                    dst = apre[j][:, t0:t0 + ch]
                    if npos == HID:
                        nc.vector.tensor_reduce(
                            dst, hrelu, axis=mybir.AxisListType.X, op=OP.add)
                    elif npos == 0:
                        nc.vector.tensor_reduce(
                            dst, hrelu, axis=mybir.AxisListType.X, op=OP.add)
                        nc.vector.tensor_scalar_mul(dst, dst, -1.0)
                    else:
                        nc.vector.tensor_reduce(
                            dst, hrelu[:, :, 0:npos],
                            axis=mybir.AxisListType.X, op=OP.add)
                        neg = hwork.tile([128, ch], fp32, tag="neg")
                        nc.vector.tensor_reduce(
                            neg, hrelu[:, :, npos:HID],
                            axis=mybir.AxisListType.X, op=OP.add)
                        nc.vector.tensor_tensor(dst, dst, neg,
                                                op=OP.subtract)
                    t0 += ch
                    while next_blk + TB <= t0:
                        do_block(j, next_blk)
                        next_blk += TB

    nc.finalize()
    return nc


def _get_program(npos):
    if npos not in _CACHE:
        _CACHE[npos] = _build_program(npos)
    return _CACHE[npos]


def kernel(r, feat, W1, b1, W2, b2, _run_kwargs=None, _return_results=False):
    from concourse.bass_utils import run_bass_kernel_spmd

    fdt = _np_feat_dtype()
    r = np.asarray(r, dtype=np.float32)
    feat = np.asarray(feat, dtype=np.float32)
    W1 = np.asarray(W1, dtype=np.float32)
    b1 = np.asarray(b1, dtype=np.float32).reshape(HID)
    W2 = np.asarray(W2, dtype=np.float32).reshape(HID)
    b2 = np.asarray(b2, dtype=np.float32).reshape(1)

    # fold W2 into W1/b1: relu(x)*w2 = sign(w2)*relu(x*|w2|); reorder
    # hidden columns so positive-sign ones come first.
    s = np.abs(W2)
    neg_mask = W2 < 0
    order = np.argsort(neg_mask, kind="stable")
    npos = int((~neg_mask).sum())
    W1f = (W1 * s[None, :])[:, order] * WSCALE
    b1f = (b1 * s)[order] * WSCALE

    # host-side downcast + transpose: [T,B,F] -> [core, j, f, t_loc, b]
    featT = np.ascontiguousarray(
        feat.astype(fdt).reshape(NCORES, TLOC, NH, 128, FEAT)
            .transpose(0, 2, 4, 1, 3))
    # r: [T,B,1] -> [core, j, b, t_loc]
    rT = np.ascontiguousarray(
        r[:, :, 0].reshape(NCORES, TLOC, NH, 128).transpose(0, 2, 3, 1))
    w1c = np.ascontiguousarray(W1f.astype(fdt))

    b1rep = np.ascontiguousarray(np.tile(b1f, BANK).astype(np.float16))
    b2rep = np.ascontiguousarray(np.full(128, b2[0], dtype=np.float32))

    nc = _get_program(npos)
    in_maps = []
    for c in range(NCORES):
        in_maps.append({
            "featT": featT[c], "rT": rT[c],
            "w1": w1c, "b1rep": b1rep, "b2rep": b2rep,
        })

    kw = _run_kwargs or {}
    res = run_bass_kernel_spmd(nc, in_maps, core_ids=list(range(NCORES)), **kw)

    # host stitch: y = z + P*carry per slab, carry chain across slabs
    y = np.empty((T, B), dtype=np.float32)
    carry = r[0, :, 0].astype(np.float32)
    for c in range(NCORES):
        zc = res.results[c]["z"].astype(np.float32).transpose(2, 0, 1)
        pc = res.results[c]["p"].astype(np.float32).transpose(2, 0, 1)
        zc = zc.reshape(TLOC, B)
        pc = pc.reshape(TLOC, B)
        y_slab = zc + pc * carry[None, :]
        carry = y_slab[-1]
        y[c * TLOC:(c + 1) * TLOC] = y_slab
    out = y[:, :, None]
    if _return_results:
        return out, res
    return out


# revision 22
# speedup vs baseline: 1.1624x; 1.0661x over previous
"""EMA head kernel for Trainium2 (Bass/Tile), 8 NeuronCores.

Problem: alpha = clip(sigmoid(MLP(feat)), 0.01, 0.99) per (t, b);
         y[0] = r[0]; y[t] = (1-alpha[t])*y[t-1] + alpha[t]*r[t].

Sharding: time dim T=4096 split into 8 slabs of 512 (all B=256 per core).
Each core computes, for its slab, the local affine-scan pieces
    z[t] = A[t]*z[t-1] + Bv[t]   (z[-1] = 0),   A = 1-alpha, Bv = alpha*r
    P[t] = A[t]*P[t-1]           (P[-1] = 1)
and the host stitches slabs with   y = z + P * carry,  carry' = y[-1].

v7 design notes (measured on this platform):
- HBM aggregate across the 8 concurrent cores saturates at ~1.5 TB/s
  (~190 GB/s/core), independent of descriptor size / queue count, so the
  16 MB/core fp8 feat stream is a hard ~85 us floor; everything else
  must hide under it.
- feat is pre-transposed + quantized to fp8 e4m3 on the host:
  featT [2 (b-half j), 128 (f), TLOC (t), 128 (b)].  Uniform 1 MB chunks
  (64 t) with a DEDICATED SBUF tile per chunk: all DMAs issue up front,
  no buffer-reuse stalls, PE starts after ~5 us and never lags.
- W2 is folded into W1/b1 on the host (relu(x)*|w2| = relu(x*|w2|),
  columns permuted positives-first) so layer 2 is just two sub-range
  reduces and a subtract -- no elementwise multiply pass.
- b1 is pre-added into PSUM with a K=1 ones-matmul per 2-bank group
  (one per chunk), so ACT's relu reads matmul output directly.
- alpha/scan tail is processed in 128-t blocks with chained
  tensor_tensor_scan; z/P written out as fp16.
"""

import numpy as np

T, B, FEAT, HID = 4096, 256, 128, 16
NCORES = 8
TLOC = T // NCORES  # 512
NH = 2              # batch halves of 128
# per-half feat chunk schedule: small first chunk so PE starts early,
# big 2MB middle chunks for descriptor efficiency (16KB/partition),
# small final chunks to shrink the serial pipeline tail.
CHUNKS = [64, 128, 128, 128, 32, 32]
BANK = 32           # t-slots per PSUM bank
TB = 128            # t-steps per alpha/scan block

FEAT_FP8 = True
# W1 columns scaled by |W2| land in fp8-e4m3's subnormal range (<2^-6);
# pre-scale by 64 on the host and undo via the sigmoid's scale param.
WSCALE = 64.0

_CACHE = {}


def _np_feat_dtype():
    if FEAT_FP8:
        import ml_dtypes
        return ml_dtypes.float8_e4m3
    return np.float16


def _build_program(npos):
    import concourse.bacc as bacc
    import concourse.bass as bass
    import concourse.tile as tile
    from concourse import mybir

    fp32 = mybir.dt.float32
    fp16 = mybir.dt.float16
    fdt = mybir.dt.float8e4 if FEAT_FP8 else mybir.dt.float16
    AF = mybir.ActivationFunctionType
    OP = mybir.AluOpType

    nc = bacc.Bacc("TRN2", target_bir_lowering=False, debug=False,
                   num_devices=NCORES)

    featT_d = nc.dram_tensor("featT", [NH, FEAT, TLOC, 128], fdt,
                             kind="ExternalInput")
    rT_d = nc.dram_tensor("rT", [NH, 128, TLOC], fp32, kind="ExternalInput")
    w1_d = nc.dram_tensor("w1", [FEAT, HID], fdt, kind="ExternalInput")
    # b1 arrives pre-folded, pre-replicated CH times, already fp16 (plain
    # DMA -- HWDGE cannot cast); b2 pre-replicated to 128 partitions.
    b1_d = nc.dram_tensor("b1rep", [BANK * HID], fp16, kind="ExternalInput")
    b2_d = nc.dram_tensor("b2rep", [128], fp32, kind="ExternalInput")
    z_d = nc.dram_tensor("z", [NH, 128, TLOC], fp16, kind="ExternalOutput")
    p_d = nc.dram_tensor("p", [NH, 128, TLOC], fp16, kind="ExternalOutput")

    with tile.TileContext(nc) as tc:
        with (
            tc.tile_pool(name="singles", bufs=1) as singles,
            tc.tile_pool(name="hps", bufs=8, space="PSUM") as hps,
            tc.tile_pool(name="hwork", bufs=3) as hwork,
            tc.tile_pool(name="apool", bufs=2) as apool,
        ):
            # feat chunk DMAs first: dedicated tiles, all issued up front
            # on the two big queues (sync HWDGE / gpsimd SWDGE).
            ft_tiles = {}
            parity = 0
            for j in range(NH):
                t0 = 0
                for ci, ch in enumerate(CHUNKS):
                    ft = singles.tile([128, ch, 128], fdt, tag=f"ft{j}_{ci}",
                                      name=f"ft{j}_{ci}")
                    eng = nc.sync if parity == 0 else nc.gpsimd
                    parity ^= 1
                    eng.dma_start(ft, featT_d[j, :, t0:t0 + ch, :])
                    ft_tiles[(j, ci)] = ft
                    t0 += ch

            # constants on the scalar (ACT) HWDGE queue -- nothing big
            # ever queues there, so they land in the first microseconds.
            w1_sb = singles.tile([128, HID], fdt)
            nc.scalar.dma_start(w1_sb, w1_d[:, :])
            b1row = singles.tile([1, BANK, HID], fp16)
            nc.scalar.dma_start(
                b1row, bass.AP(b1_d, 0, [[0, 1], [HID, BANK], [1, HID]]))
            b2col = singles.tile([128, 1], fp32)
            nc.scalar.dma_start(b2col, bass.AP(b2_d, 0, [[1, 128], [1, 1]]))
            rT = [singles.tile([128, TLOC], fp32, tag=f"rT{h}", name=f"rT{h}")
                  for h in range(NH)]
            for h in range(NH):
                nc.scalar.dma_start(rT[h], rT_d[h])
            ones1 = singles.tile([1, 128], fp16)
            nc.vector.memset(ones1, 1.0)
            ones_tb = singles.tile([128, TB], fp32)
            nc.vector.memset(ones_tb, 1.0)

            apre = [singles.tile([128, TLOC], fp32, tag=f"apre{h}",
                                 name=f"apre{h}")
                    for h in range(NH)]
            z_sb = [singles.tile([128, TLOC], fp16, tag=f"z{h}", name=f"z{h}")
                    for h in range(NH)]
            p_sb = [singles.tile([128, TLOC], fp16, tag=f"p{h}", name=f"p{h}")
                    for h in range(NH)]

            def do_block(j, blk):
                """alpha -> A,Bv -> chained scans for t in [blk, blk+TB)."""
                al = apool.tile([128, TB], fp32, tag="al")
                nc.scalar.activation(al, apre[j][:, blk:blk + TB],
                                     AF.Sigmoid, bias=b2col,
                                     scale=1.0 / WSCALE)
                nc.vector.tensor_scalar(al, al, 0.01, 0.99,
                                        op0=OP.max, op1=OP.min)
                A_b = apool.tile([128, TB], fp32, tag="A")
                nc.vector.tensor_scalar(A_b, al, -1.0, 1.0,
                                        op0=OP.mult, op1=OP.add)
                Bv = apool.tile([128, TB], fp32, tag="Bv")
                nc.vector.tensor_mul(Bv, al, rT[j][:, blk:blk + TB])
                z0 = 0.0 if blk == 0 else z_sb[j][:, blk - 1:blk]
                nc.vector.tensor_tensor_scan(
                    z_sb[j][:, blk:blk + TB], A_b, Bv, z0,
                    op0=OP.mult, op1=OP.add)
                p0 = 1.0 if blk == 0 else p_sb[j][:, blk - 1:blk]
                nc.vector.tensor_tensor_scan(
                    p_sb[j][:, blk:blk + TB], A_b, ones_tb, p0,
                    op0=OP.mult, op1=OP.mult)
                done = blk + TB
                if done == TLOC // 2 or done == TLOC:
                    lo = 0 if done == TLOC // 2 else TLOC // 2
                    nc.gpsimd.dma_start(z_d[j][:, lo:done],
                                        z_sb[j][:, lo:done])
                    nc.sync.dma_start(p_d[j][:, lo:done],
                                      p_sb[j][:, lo:done])

            for j in range(NH):
                t0 = 0
                next_blk = 0
                for ci, ch in enumerate(CHUNKS):
                    ft = ft_tiles[(j, ci)]
                    # one 2-bank PSUM group per chunk: bias preload via
                    # K=1 ones-matmul, then 64 accumulating per-t matmuls
                    # per-BANK pipeline, 8 PSUM banks deep: preload
                    # bias (K=1 ones-matmul; matmul output may not cross
                    # the 512-fp32 bank boundary), 32 accumulating per-t
                    # matmuls, relu to fp16, then the folded-W2 layer 2:
                    # sum positive-sign columns minus sum of negatives.
                    for hb in range(ch // BANK):
                        hbank = hps.tile([128, BANK, HID], fp32, tag="hb")
                        nc.tensor.matmul(hbank, ones1, b1row,
                                         start=True, stop=False)
                        for si in range(BANK):
                            s = hb * BANK + si
                            nc.tensor.matmul(hbank[:, si, :], ft[:, s, :],
                                             w1_sb, start=False,
                                             stop=(si == BANK - 1))
                        hrelu = hwork.tile([128, BANK, HID], fp16,
                                           tag="hrelu")
                        nc.scalar.activation(hrelu, hbank, AF.Relu)
                        b0 = t0 + hb * BANK
                        dst = apre[j][:, b0:b0 + BANK]
                        if npos == HID:
                            nc.vector.tensor_reduce(
                                dst, hrelu, axis=mybir.AxisListType.X,
                                op=OP.add)
                        elif npos == 0:
                            nc.vector.tensor_reduce(
                                dst, hrelu, axis=mybir.AxisListType.X,
                                op=OP.add)
                            nc.vector.tensor_scalar_mul(dst, dst, -1.0)
                        else:
                            nc.vector.tensor_reduce(
                                dst, hrelu[:, :, 0:npos],
                                axis=mybir.AxisListType.X, op=OP.add)
                            neg = hwork.tile([128, BANK], fp32, tag="neg")
                            nc.vector.tensor_reduce(
                                neg, hrelu[:, :, npos:HID],
                                axis=mybir.AxisListType.X, op=OP.add)
                            nc.vector.tensor_tensor(dst, dst, neg,
                                                    op=OP.subtract)
                    t0 += ch
                    while next_blk + TB <= t0:
                        do_block(j, next_blk)
                        next_blk += TB

    nc.finalize()
    return nc


def _get_program(npos):
    if npos not in _CACHE:
        _CACHE[npos] = _build_program(npos)
    return _CACHE[npos]


def kernel(r, feat, W1, b1, W2, b2, _run_kwargs=None, _return_results=False):
    from concourse.bass_utils import run_bass_kernel_spmd

    fdt = _np_feat_dtype()
    r = np.asarray(r, dtype=np.float32)
    feat = np.asarray(feat, dtype=np.float32)
    W1 = np.asarray(W1, dtype=np.float32)
    b1 = np.asarray(b1, dtype=np.float32).reshape(HID)
    W2 = np.asarray(W2, dtype=np.float32).reshape(HID)
    b2 = np.asarray(b2, dtype=np.float32).reshape(1)

    # fold W2 into W1/b1: relu(x)*w2 = sign(w2)*relu(x*|w2|); reorder
    # hidden columns so positive-sign ones come first.
    s = np.abs(W2)
    neg_mask = W2 < 0
    order = np.argsort(neg_mask, kind="stable")
    npos = int((~neg_mask).sum())
    W1f = (W1 * s[None, :])[:, order] * WSCALE
    b1f = (b1 * s)[order] * WSCALE

    # host-side downcast + transpose: [T,B,F] -> [core, j, f, t_loc, b]
    featT = np.ascontiguousarray(
        feat.astype(fdt).reshape(NCORES, TLOC, NH, 128, FEAT)
            .transpose(0, 2, 4, 1, 3))
    # r: [T,B,1] -> [core, j, b, t_loc]
    rT = np.ascontiguousarray(
        r[:, :, 0].reshape(NCORES, TLOC, NH, 128).transpose(0, 2, 3, 1))
    w1c = np.ascontiguousarray(W1f.astype(fdt))

    b1rep = np.ascontiguousarray(np.tile(b1f, BANK).astype(np.float16))
    b2rep = np.ascontiguousarray(np.full(128, b2[0], dtype=np.float32))

    nc = _get_program(npos)
    in_maps = []
    for c in range(NCORES):
        in_maps.append({
            "featT": featT[c], "rT": rT[c],
            "w1": w1c, "b1rep": b1rep, "b2rep": b2rep,
        })

    kw = _run_kwargs or {}
    res = run_bass_kernel_spmd(nc, in_maps, core_ids=list(range(NCORES)), **kw)

    # host stitch: y = z + P*carry per slab, carry chain across slabs
    y = np.empty((T, B), dtype=np.float32)
    carry = r[0, :, 0].astype(np.float32)
    for c in range(NCORES):
        zc = res.results[c]["z"].astype(np.float32).transpose(2, 0, 1)
        pc = res.results[c]["p"].astype(np.float32).transpose(2, 0, 1)
        zc = zc.reshape(TLOC, B)
        pc = pc.reshape(TLOC, B)
        y_slab = zc + pc * carry[None, :]
        carry = y_slab[-1]
        y[c * TLOC:(c + 1) * TLOC] = y_slab
    out = y[:, :, None]
    if _return_results:
        return out, res
    return out
